# revision 5
# baseline (speedup 1.0000x reference)
"""Bahdanau additive attention kernel for 8 TRN2 NeuronCores.

Reference math (per batch b):
    c = context @ Wc.T                     (L1, D)
    a = aspect  @ Wa.T                     (L2, D)
    scores[i,j] = sum_d V[d] * tanh(c[i,d] + a[j,d])
    alpha = softmax_j(scores)
    out = alpha @ aspect                   (L1, D)

Sharding: data-parallel over batch, 4 batches per core, no collectives.

Device mapping (per batch):
  - projections cT = Wc @ ctxT and aT = Wa @ aspT with the contraction (input
    feature) dim on partitions; host pre-transposes all operands so no
    on-device transposes are needed.
  - main loop over 4 e-chunks (output feature dim, 128 partitions each):
      DVE tensor_scalar_add broadcasts aT[:, j] over the i dim  (bf16, 4x mode)
      ACT computes one big tanh over a [128, JH*256] slab
      PE reduces against V with a "sliding diagonal" stationary operand:
        lhsT = vdiag[:, 63-j : 127-j]  (V in column j, zeros elsewhere)
        each matmul accumulates scoresT[j, :] into a [64, 256] PSUM tile
  - epilogue: exp on ACT (no max subtraction needed: |scores| <= sum|V| ~ 18),
    row sums + alpha@aspect as K=64 matmuls, final normalize by reciprocal
    on the way out.
"""

import numpy as np
import ml_dtypes

B, L1, L2, D = 32, 256, 64, 512
NCORES = 8
NB = B // NCORES          # batches per core
P = 128                   # partitions
NCH = D // P              # feature chunks (4)
JH = 64                   # j-slab size (1 slab of 64 per e-chunk)
GP_MOD = 4                # every GP_MOD-th broadcast-add goes to GpSimd
NI = L1 // P              # i chunks (2)

BF16 = ml_dtypes.bfloat16

_CACHE = {}


def _build():
    import concourse.bass as bass
    import concourse.tile as tile
    from concourse import bacc, mybir

    f32 = mybir.dt.float32
    bf16 = mybir.dt.bfloat16
    AFT = mybir.ActivationFunctionType
    ts = bass.ts

    nc = bacc.Bacc("TRN2", target_bir_lowering=False, debug=False,
                   num_devices=NCORES)

    ctxT_d = nc.dram_tensor("ctxT", [NB, P, NCH, L1], bf16, kind="ExternalInput")
    aspT_d = nc.dram_tensor("aspT", [NB, P, NCH, L2], bf16, kind="ExternalInput")
    asp_d = nc.dram_tensor("asp", [NB, L2, D], bf16, kind="ExternalInput")
    WcT_d = nc.dram_tensor("WcT", [P, NCH, D], bf16, kind="ExternalInput")
    WaT_d = nc.dram_tensor("WaT", [P, NCH, D], bf16, kind="ExternalInput")
    vdiag_d = nc.dram_tensor("vdiag", [P, NCH, 2 * L2 - 1], bf16, kind="ExternalInput")
    out_d = nc.dram_tensor("out", [NB, L1, D], f32, kind="ExternalOutput")

    with tile.TileContext(nc) as tc:
        with (
            tc.tile_pool(name="wpool", bufs=1) as wpool,
            tc.tile_pool(name="inpool", bufs=2) as inpool,
            tc.tile_pool(name="proj", bufs=2, space=bass.MemorySpace.PSUM) as projp,
            tc.tile_pool(name="ctpool", bufs=2) as ctpool,
            tc.tile_pool(name="slab", bufs=2) as slabp,
            tc.tile_pool(name="scores", bufs=2, space=bass.MemorySpace.PSUM) as scoresp,
            tc.tile_pool(name="eps", bufs=2, space=bass.MemorySpace.PSUM) as epsp,
            tc.tile_pool(name="small", bufs=2) as smallp,
            tc.tile_pool(name="epool", bufs=2) as epool,
            tc.tile_pool(name="outp", bufs=2) as outpool,
        ):
            WcT = wpool.tile([P, NCH, D], bf16)
            WaT = wpool.tile([P, NCH, D], bf16)
            vdiag = wpool.tile([P, NCH, 2 * L2 - 1], bf16)
            ones = wpool.tile([L2, 1], bf16)
            nc.sync.dma_start(WcT[:], WcT_d[:])
            nc.sync.dma_start(WaT[:], WaT_d[:])
            nc.sync.dma_start(vdiag[:], vdiag_d[:])
            nc.gpsimd.memset(ones[:], 1.0)

            for b in range(NB):
                ctxT = inpool.tile([P, NCH, L1], bf16, tag="ctx")
                aspT = inpool.tile([P, NCH, L2], bf16, tag="aspT")
                asp = inpool.tile([L2, D], bf16, tag="asp")
                nc.sync.dma_start(ctxT[:], ctxT_d[b])
                nc.sync.dma_start(aspT[:], aspT_d[b])
                nc.sync.dma_start(asp[:], asp_d[b])

                # projections: cT[e,i] = sum_d WcT[d,e] * ctxT[d,i]
                cT = ctpool.tile([P, NCH, L1], bf16, tag="ct")
                for m in range(NCH):
                    ps = projp.tile([P, L1], f32, tag="proj")
                    for c in range(NCH):
                        nc.tensor.matmul(ps[:], WcT[:, c, ts(m, P)], ctxT[:, c, :],
                                         start=(c == 0), stop=(c == NCH - 1))
                    nc.vector.tensor_copy(cT[:, m, :], ps[:])
                aT = ctpool.tile([P, NCH, L2], f32, tag="at")
                for m in range(NCH):
                    ps = projp.tile([P, L2], f32, tag="proj")
                    for c in range(NCH):
                        nc.tensor.matmul(ps[:], WaT[:, c, ts(m, P)], aspT[:, c, :],
                                         start=(c == 0), stop=(c == NCH - 1))
                    nc.vector.tensor_copy(aT[:, m, :], ps[:])

                # scoresT[j, i] accumulated over all 4 e-chunks
                scores = scoresp.tile([L2, L1], f32)
                for m in range(NCH):
                    for h in range(L2 // JH):
                        tmp = slabp.tile([P, JH, L1], bf16, tag="tmp")
                        for j in range(JH):
                            jj = h * JH + j
                            eng = nc.gpsimd if (j % GP_MOD == GP_MOD - 1) else nc.vector
                            eng.tensor_scalar_add(
                                tmp[:, j, :], cT[:, m, :], aT[:, m, jj:jj + 1])
                        tha = slabp.tile([P, JH, L1], bf16, tag="tanh")
                        nc.scalar.activation(tha[:], tmp[:], AFT.Tanh)
                        for j in range(JH):
                            jj = h * JH + j
                            nc.tensor.matmul(
                                scores[:],
                                vdiag[:, m, L2 - 1 - jj:2 * L2 - 1 - jj],
                                tha[:, j, :],
                                start=(m == 0 and jj == 0),
                                stop=(m == NCH - 1 and jj == L2 - 1))

                # softmax (over j = partitions of scores) + weighted sum
                E = epool.tile([L2, L1], bf16)
                nc.scalar.activation(E[:], scores[:], AFT.Exp)
                for i in range(NI):
                    sums = epsp.tile([P, 1], f32, tag="eps")
                    nc.tensor.matmul(sums[:], E[:, ts(i, P)], ones[:])
                    recip = smallp.tile([P, 1], f32)
                    nc.vector.reciprocal(recip[:], sums[:])
                    op = epsp.tile([P, D], f32, tag="eps")
                    nc.tensor.matmul(op[:], E[:, ts(i, P)], asp[:])
                    osb = outpool.tile([P, D], f32)
                    nc.vector.tensor_scalar_mul(osb[:], op[:], recip[:])
                    nc.sync.dma_start(out_d[b, ts(i, P), :], osb[:])

    nc.compile()
    return nc


def _get_nc():
    if "nc" not in _CACHE:
        _CACHE["nc"] = _build()
    return _CACHE["nc"]


def _shard_inputs(context, aspect, Wc, Wa, V):
    """Host-side preprocessing: shard over batch, transpose + cast to bf16."""
    context = np.asarray(context)
    aspect = np.asarray(aspect)
    Wc = np.asarray(Wc)
    Wa = np.asarray(Wa)
    V = np.asarray(V)

    # [p, c, e] = W[e, c*128+p]
    def wt(W):
        return np.ascontiguousarray(
            W.T.reshape(NCH, P, D).transpose(1, 0, 2)).astype(BF16)

    WcT = wt(Wc)
    WaT = wt(Wa)
    vdiag = np.zeros((P, NCH, 2 * L2 - 1), dtype=BF16)
    vdiag[:, :, L2 - 1] = V.reshape(NCH, P).T.astype(BF16)

    in_maps = []
    for k in range(NCORES):
        ctx_s = context[NB * k:NB * (k + 1)]   # (NB, L1, D)
        asp_s = aspect[NB * k:NB * (k + 1)]    # (NB, L2, D)
        # [b, p, c, i] = ctx[b, i, c*128+p]
        ctxT = np.ascontiguousarray(
            ctx_s.transpose(0, 2, 1).reshape(NB, NCH, P, L1).transpose(0, 2, 1, 3)
        ).astype(BF16)
        aspT = np.ascontiguousarray(
            asp_s.transpose(0, 2, 1).reshape(NB, NCH, P, L2).transpose(0, 2, 1, 3)
        ).astype(BF16)
        in_maps.append({
            "ctxT": ctxT,
            "aspT": aspT,
            "asp": asp_s.astype(BF16),
            "WcT": WcT,
            "WaT": WaT,
            "vdiag": vdiag,
        })
    return in_maps


def run(inputs, trace=False, trace_kwargs=None, tmpdir=None):
    """Run on all 8 cores. Returns (full_output, BassKernelResults)."""
    from concourse.bass_utils import run_bass_kernel_spmd

    nc = _get_nc()
    in_maps = _shard_inputs(**inputs)
    res = run_bass_kernel_spmd(
        nc, in_maps, core_ids=list(range(NCORES)),
        trace=trace, trace_kwargs=trace_kwargs or {}, tmpdir=tmpdir)
    out = np.concatenate([res.results[k]["out"] for k in range(NCORES)], axis=0)
    return out.astype(np.float32), res


def kernel(**inputs):
    return run(inputs)[0]


# revision 9
# speedup vs baseline: 4.1207x; 4.1207x over previous
"""Bahdanau additive attention kernel for 8 TRN2 NeuronCores.

Reference math (per batch b):
    c = context @ Wc.T                     (L1, D)
    a = aspect  @ Wa.T                     (L2, D)
    scores[i,j] = sum_d V[d] * tanh(c[i,d] + a[j,d])
    alpha = softmax_j(scores)
    out = alpha @ aspect                   (L1, D)

Sharding: data-parallel over batch, 4 batches per core, no collectives.

Device mapping (per batch):
  - projections cT = Wc @ ctxT and aT = Wa @ aspT with the contraction (input
    feature) dim on partitions; host pre-transposes all operands so no
    on-device transposes are needed.
  - main loop over 4 e-chunks (output feature dim, 128 partitions each):
      DVE tensor_scalar_add broadcasts aT[:, j] over the i dim  (bf16, 4x mode)
      ACT computes one big tanh over a [128, JH*256] slab
      PE reduces against V with a "sliding diagonal" stationary operand:
        lhsT = vdiag[:, 63-j : 127-j]  (V in column j, zeros elsewhere)
        each matmul accumulates scoresT[j, :] into a [64, 256] PSUM tile
  - epilogue: exp on ACT (no max subtraction needed: |scores| <= sum|V| ~ 18),
    row sums + alpha@aspect as K=64 matmuls, final normalize by reciprocal
    on the way out.
"""

import numpy as np
import ml_dtypes

B, L1, L2, D = 32, 256, 64, 512
NCORES = 8
NB = B // NCORES          # batches per core
P = 128                   # partitions
NCH = D // P              # feature chunks (4)
JH = 64                   # j-slab size (1 slab of 64 per e-chunk)
NI = L1 // P              # i chunks (2)

BF16 = ml_dtypes.bfloat16

_CACHE = {}


def _build():
    import concourse.bass as bass
    import concourse.tile as tile
    from concourse import bacc, mybir

    f32 = mybir.dt.float32
    bf16 = mybir.dt.bfloat16
    AFT = mybir.ActivationFunctionType
    ts = bass.ts

    nc = bacc.Bacc("TRN2", target_bir_lowering=False, debug=False,
                   num_devices=NCORES)

    ctxT_d = nc.dram_tensor("ctxT", [NB, P, NCH, L1], bf16, kind="ExternalInput")
    aspT_d = nc.dram_tensor("aspT", [NB, P, NCH, L2], bf16, kind="ExternalInput")
    asp_d = nc.dram_tensor("asp", [NB, L2, D], bf16, kind="ExternalInput")
    WcT_d = nc.dram_tensor("WcT", [P, NCH, D], bf16, kind="ExternalInput")
    WaT_d = nc.dram_tensor("WaT", [P, NCH, D], bf16, kind="ExternalInput")
    vdiag_d = nc.dram_tensor("vdiag", [P, NCH, 2 * L2 - 1], bf16, kind="ExternalInput")
    out_d = nc.dram_tensor("out", [NB, L1, D], f32, kind="ExternalOutput")

    with tile.TileContext(nc) as tc:
        with (
            tc.tile_pool(name="wpool", bufs=1) as wpool,
            tc.tile_pool(name="inpool", bufs=2) as inpool,
            tc.tile_pool(name="proj", bufs=1, space=bass.MemorySpace.PSUM) as projp,
            tc.tile_pool(name="ctpool", bufs=2) as ctpool,
            tc.tile_pool(name="slab", bufs=2) as slabp,
            tc.tile_pool(name="scores", bufs=2, space=bass.MemorySpace.PSUM) as scoresp,
            tc.tile_pool(name="eps", bufs=2, space=bass.MemorySpace.PSUM) as epsp,
            tc.tile_pool(name="small", bufs=2) as smallp,
            tc.tile_pool(name="epool", bufs=2) as epool,
            tc.tile_pool(name="outp", bufs=2) as outpool,
        ):
            WcT = wpool.tile([P, NCH, D], bf16)
            WaT = wpool.tile([P, NCH, D], bf16)
            vdiag = wpool.tile([P, NCH, 2 * L2 - 1], bf16)
            ones = wpool.tile([L2, 1], bf16)
            scratch = wpool.tile([L2, 1], bf16)
            nc.sync.dma_start(WcT[:], WcT_d[:])
            nc.sync.dma_start(WaT[:], WaT_d[:])
            nc.sync.dma_start(vdiag[:], vdiag_d[:])
            nc.gpsimd.memset(ones[:], 1.0)
            # warm up the ACT table set (tanh/exp share one set) during DMAs
            nc.scalar.activation(scratch[:], ones[:], AFT.Tanh)

            for b in range(NB):
                ctxT = inpool.tile([P, NCH, L1], bf16, tag="ctx")
                aspT = inpool.tile([P, NCH, L2], bf16, tag="aspT")
                asp = inpool.tile([L2, D], bf16, tag="asp")
                nc.sync.dma_start(ctxT[:], ctxT_d[b])
                nc.sync.dma_start(aspT[:], aspT_d[b])
                nc.sync.dma_start(asp[:], asp_d[b])

                # projections: cT[e,i] = sum_d WcT[d,e] * ctxT[d,i]
                # single shared PSUM tiles (2 banks for c, 1 for a), one
                # wide cast each -- except b=0 where per-chunk casts let the
                # first slab start sooner.
                cT = ctpool.tile([P, NCH, L1], bf16, tag="ct")
                aT = ctpool.tile([P, NCH, L2], f32, tag="at")
                psc = projp.tile([P, NCH, L1], f32, tag="projc")
                psa = projp.tile([P, NCH, L2], f32, tag="proja")
                for m in range(NCH):
                    for c in range(NCH):
                        nc.tensor.matmul(psc[:, m, :], WcT[:, c, ts(m, P)],
                                         ctxT[:, c, :],
                                         start=(c == 0), stop=(c == NCH - 1))
                    for c in range(NCH):
                        nc.tensor.matmul(psa[:, m, :], WaT[:, c, ts(m, P)],
                                         aspT[:, c, :],
                                         start=(c == 0), stop=(c == NCH - 1))
                    if b == 0:
                        nc.vector.tensor_copy(cT[:, m, :], psc[:, m, :])
                        nc.vector.tensor_copy(aT[:, m, :], psa[:, m, :])
                if b != 0:
                    nc.vector.tensor_copy(cT[:], psc[:])
                    nc.vector.tensor_copy(aT[:], psa[:])

                # scoresT[j, i] accumulated over all 4 e-chunks
                scores = scoresp.tile([L2, L1], f32)
                for m in range(NCH):
                    # first and last (b, m): smaller sub-slabs to cut ACT
                    # startup / drain-tail idle
                    if (b == 0 and m == 0) or (b == NB - 1 and m == NCH - 1):
                        subs = [16] * (L2 // 16)
                    else:
                        subs = [JH] * (L2 // JH)
                    jj = 0
                    for sub in subs:
                        tmp = slabp.tile([P, sub, L1], bf16, tag="tmp")
                        for j in range(sub):
                            nc.vector.tensor_scalar_add(
                                tmp[:, j, :], cT[:, m, :], aT[:, m, jj + j:jj + j + 1])
                        tha = slabp.tile([P, sub, L1], bf16, tag="tanh")
                        nc.scalar.activation(tha[:], tmp[:], AFT.Tanh)
                        for j in range(sub):
                            nc.tensor.matmul(
                                scores[:],
                                vdiag[:, m, L2 - 1 - (jj + j):2 * L2 - 1 - (jj + j)],
                                tha[:, j, :],
                                start=(m == 0 and jj + j == 0),
                                stop=(m == NCH - 1 and jj + j == L2 - 1))
                        jj += sub

                # softmax (over j = partitions of scores) + weighted sum
                E = epool.tile([L2, L1], bf16)
                nc.scalar.activation(E[:], scores[:], AFT.Exp)
                for i in range(NI):
                    sums = epsp.tile([P, 1], f32, tag="eps")
                    nc.tensor.matmul(sums[:], E[:, ts(i, P)], ones[:])
                    recip = smallp.tile([P, 1], f32)
                    nc.vector.reciprocal(recip[:], sums[:])
                    op = epsp.tile([P, D], f32, tag="eps")
                    nc.tensor.matmul(op[:], E[:, ts(i, P)], asp[:])
                    osb = outpool.tile([P, D], f32)
                    nc.vector.tensor_scalar_mul(osb[:], op[:], recip[:])
                    nc.sync.dma_start(out_d[b, ts(i, P), :], osb[:])

    nc.compile()
    return nc


def _get_nc():
    if "nc" not in _CACHE:
        _CACHE["nc"] = _build()
    return _CACHE["nc"]


def _shard_inputs(context, aspect, Wc, Wa, V):
    """Host-side preprocessing: shard over batch, transpose + cast to bf16."""
    context = np.asarray(context)
    aspect = np.asarray(aspect)
    Wc = np.asarray(Wc)
    Wa = np.asarray(Wa)
    V = np.asarray(V)

    # [p, c, e] = W[e, c*128+p]
    def wt(W):
        return np.ascontiguousarray(
            W.T.reshape(NCH, P, D).transpose(1, 0, 2)).astype(BF16)

    WcT = wt(Wc)
    WaT = wt(Wa)
    vdiag = np.zeros((P, NCH, 2 * L2 - 1), dtype=BF16)
    vdiag[:, :, L2 - 1] = V.reshape(NCH, P).T.astype(BF16)

    in_maps = []
    for k in range(NCORES):
        ctx_s = context[NB * k:NB * (k + 1)]   # (NB, L1, D)
        asp_s = aspect[NB * k:NB * (k + 1)]    # (NB, L2, D)
        # [b, p, c, i] = ctx[b, i, c*128+p]
        ctxT = np.ascontiguousarray(
            ctx_s.transpose(0, 2, 1).reshape(NB, NCH, P, L1).transpose(0, 2, 1, 3)
        ).astype(BF16)
        aspT = np.ascontiguousarray(
            asp_s.transpose(0, 2, 1).reshape(NB, NCH, P, L2).transpose(0, 2, 1, 3)
        ).astype(BF16)
        in_maps.append({
            "ctxT": ctxT,
            "aspT": aspT,
            "asp": asp_s.astype(BF16),
            "WcT": WcT,
            "WaT": WaT,
            "vdiag": vdiag,
        })
    return in_maps


def run(inputs, trace=False, trace_kwargs=None, tmpdir=None):
    """Run on all 8 cores. Returns (full_output, BassKernelResults)."""
    from concourse.bass_utils import run_bass_kernel_spmd

    nc = _get_nc()
    in_maps = _shard_inputs(**inputs)
    res = run_bass_kernel_spmd(
        nc, in_maps, core_ids=list(range(NCORES)),
        trace=trace, trace_kwargs=trace_kwargs or {}, tmpdir=tmpdir)
    out = np.concatenate([res.results[k]["out"] for k in range(NCORES)], axis=0)
    return out.astype(np.float32), res


def kernel(**inputs):
    return run(inputs)[0]


# revision 12
# speedup vs baseline: 4.2340x; 1.0275x over previous
"""Bahdanau additive attention kernel for 8 TRN2 NeuronCores.

Reference math (per batch b):
    c = context @ Wc.T                     (L1, D)
    a = aspect  @ Wa.T                     (L2, D)
    scores[i,j] = sum_d V[d] * tanh(c[i,d] + a[j,d])
    alpha = softmax_j(scores)
    out = alpha @ aspect                   (L1, D)

Sharding: data-parallel over batch, 4 batches per core, no collectives.

Device mapping (per batch):
  - projections cT = Wc @ ctxT and aT = Wa @ aspT with the contraction (input
    feature) dim on partitions; host pre-transposes all operands so no
    on-device transposes are needed.
  - main loop over 4 e-chunks (output feature dim, 128 partitions each):
      DVE tensor_scalar_add broadcasts aT[:, j] over the i dim  (bf16, 4x mode)
      ACT computes one big tanh over a [128, JH*256] slab
      PE reduces against V with a "sliding diagonal" stationary operand:
        lhsT = vdiag[:, 63-j : 127-j]  (V in column j, zeros elsewhere)
        each matmul accumulates scoresT[j, :] into a [64, 256] PSUM tile
  - epilogue: exp on ACT (no max subtraction needed: |scores| <= sum|V| ~ 18),
    row sums + alpha@aspect as K=64 matmuls, final normalize by reciprocal
    on the way out.
"""

import numpy as np
import ml_dtypes

B, L1, L2, D = 32, 256, 64, 512
NCORES = 8
NB = B // NCORES          # batches per core
P = 128                   # partitions
NCH = D // P              # feature chunks (4)
JH = 32                   # j-slab size (2 slabs of 32 per e-chunk)
NI = L1 // P              # i chunks (2)

BF16 = ml_dtypes.bfloat16

_CACHE = {}


def _build():
    import concourse.bass as bass
    import concourse.tile as tile
    from concourse import bacc, mybir

    f32 = mybir.dt.float32
    bf16 = mybir.dt.bfloat16
    AFT = mybir.ActivationFunctionType
    ts = bass.ts

    nc = bacc.Bacc("TRN2", target_bir_lowering=False, debug=False,
                   num_devices=NCORES)

    ctxT_d = nc.dram_tensor("ctxT", [NB, P, NCH, L1], bf16, kind="ExternalInput")
    aspT_d = nc.dram_tensor("aspT", [NB, P, NCH, L2], bf16, kind="ExternalInput")
    asp_d = nc.dram_tensor("asp", [NB, L2, D], bf16, kind="ExternalInput")
    WcT_d = nc.dram_tensor("WcT", [P, NCH, D], bf16, kind="ExternalInput")
    WaT_d = nc.dram_tensor("WaT", [P, NCH, D], bf16, kind="ExternalInput")
    vdiag_d = nc.dram_tensor("vdiag", [P, NCH, 2 * L2 - 1], bf16, kind="ExternalInput")
    out_d = nc.dram_tensor("out", [NB, L1, D], f32, kind="ExternalOutput")

    with tile.TileContext(nc) as tc:
        with (
            tc.tile_pool(name="wpool", bufs=1) as wpool,
            tc.tile_pool(name="inpool", bufs=2) as inpool,
            tc.tile_pool(name="proj", bufs=1, space=bass.MemorySpace.PSUM) as projp,
            tc.tile_pool(name="ctpool", bufs=2) as ctpool,
            tc.tile_pool(name="slab", bufs=3) as slabp,
            tc.tile_pool(name="scores", bufs=2, space=bass.MemorySpace.PSUM) as scoresp,
            tc.tile_pool(name="eps", bufs=2, space=bass.MemorySpace.PSUM) as epsp,
            tc.tile_pool(name="small", bufs=2) as smallp,
            tc.tile_pool(name="epool", bufs=2) as epool,
            tc.tile_pool(name="outp", bufs=2) as outpool,
        ):
            WcT = wpool.tile([P, NCH, D], bf16)
            WaT = wpool.tile([P, NCH, D], bf16)
            vdiag = wpool.tile([P, NCH, 2 * L2 - 1], bf16)
            ones = wpool.tile([L2, 1], bf16)
            scratch = wpool.tile([L2, 1], bf16)
            nc.gpsimd.memset(ones[:], 1.0)
            # warm up the ACT table set (tanh/exp share one set) during DMAs
            nc.scalar.activation(scratch[:], ones[:], AFT.Tanh)

            for b in range(NB):
                ctxT = inpool.tile([P, NCH, L1], bf16, tag="ctx")
                aspT = inpool.tile([P, NCH, L2], bf16, tag="aspT")
                asp = inpool.tile([L2, D], bf16, tag="asp")
                if b == 0:
                    # chunked DMAs ordered so the first projection's operands
                    # land first -- shortens the startup critical path
                    for c in range(NCH):
                        nc.sync.dma_start(WcT[:, c, :], WcT_d[:, c, :])
                        nc.sync.dma_start(ctxT[:, c, :], ctxT_d[b, :, c, :])
                        nc.sync.dma_start(WaT[:, c, :], WaT_d[:, c, :])
                        nc.sync.dma_start(aspT[:, c, :], aspT_d[b, :, c, :])
                    nc.sync.dma_start(vdiag[:], vdiag_d[:])
                    nc.sync.dma_start(asp[:], asp_d[b])
                else:
                    nc.sync.dma_start(ctxT[:], ctxT_d[b])
                    nc.sync.dma_start(aspT[:], aspT_d[b])
                    nc.sync.dma_start(asp[:], asp_d[b])

                # projections: cT[e,i] = sum_d WcT[d,e] * ctxT[d,i]
                # single shared PSUM tiles (2 banks for c, 1 for a), one
                # wide cast each -- except b=0 where per-chunk casts let the
                # first slab start sooner.
                cT = ctpool.tile([P, NCH, L1], bf16, tag="ct")
                aT = ctpool.tile([P, NCH, L2], f32, tag="at")
                psc = projp.tile([P, NCH, L1], f32, tag="projc")
                psa = projp.tile([P, NCH, L2], f32, tag="proja")
                for m in range(NCH):
                    for c in range(NCH):
                        nc.tensor.matmul(psc[:, m, :], WcT[:, c, ts(m, P)],
                                         ctxT[:, c, :],
                                         start=(c == 0), stop=(c == NCH - 1))
                    for c in range(NCH):
                        nc.tensor.matmul(psa[:, m, :], WaT[:, c, ts(m, P)],
                                         aspT[:, c, :],
                                         start=(c == 0), stop=(c == NCH - 1))
                    if b == 0:
                        nc.vector.tensor_copy(cT[:, m, :], psc[:, m, :])
                        nc.vector.tensor_copy(aT[:, m, :], psa[:, m, :])
                if b != 0:
                    nc.vector.tensor_copy(cT[:], psc[:])
                    nc.vector.tensor_copy(aT[:], psa[:])

                # scoresT[j, i] accumulated over all 4 e-chunks
                scores = scoresp.tile([L2, L1], f32)
                for m in range(NCH):
                    # first and last (b, m): smaller sub-slabs to cut ACT
                    # startup / drain-tail idle
                    if (b == 0 and m == 0) or (b == NB - 1 and m == NCH - 1):
                        subs = [16] * (L2 // 16)
                    else:
                        subs = [JH] * (L2 // JH)
                    jj = 0
                    for sub in subs:
                        tmp = slabp.tile([P, sub, L1], bf16, tag="tmp")
                        for j in range(sub):
                            nc.vector.tensor_scalar_add(
                                tmp[:, j, :], cT[:, m, :], aT[:, m, jj + j:jj + j + 1])
                        tha = slabp.tile([P, sub, L1], bf16, tag="tanh")
                        nc.scalar.activation(tha[:], tmp[:], AFT.Tanh)
                        for j in range(sub):
                            nc.tensor.matmul(
                                scores[:],
                                vdiag[:, m, L2 - 1 - (jj + j):2 * L2 - 1 - (jj + j)],
                                tha[:, j, :],
                                start=(m == 0 and jj + j == 0),
                                stop=(m == NCH - 1 and jj + j == L2 - 1))
                        jj += sub

                # softmax (over j = partitions of scores) + weighted sum
                E = epool.tile([L2, L1], bf16)
                nc.scalar.activation(E[:], scores[:], AFT.Exp)
                for i in range(NI):
                    sums = epsp.tile([P, 1], f32, tag="eps")
                    nc.tensor.matmul(sums[:], E[:, ts(i, P)], ones[:])
                    recip = smallp.tile([P, 1], f32)
                    nc.vector.reciprocal(recip[:], sums[:])
                    op = epsp.tile([P, D], f32, tag="eps")
                    nc.tensor.matmul(op[:], E[:, ts(i, P)], asp[:])
                    osb = outpool.tile([P, D], f32)
                    nc.vector.tensor_scalar_mul(osb[:], op[:], recip[:])
                    nc.sync.dma_start(out_d[b, ts(i, P), :], osb[:])

    nc.compile()
    return nc


def _get_nc():
    if "nc" not in _CACHE:
        _CACHE["nc"] = _build()
    return _CACHE["nc"]


def _shard_inputs(context, aspect, Wc, Wa, V):
    """Host-side preprocessing: shard over batch, transpose + cast to bf16."""
    context = np.asarray(context)
    aspect = np.asarray(aspect)
    Wc = np.asarray(Wc)
    Wa = np.asarray(Wa)
    V = np.asarray(V)

    # [p, c, e] = W[e, c*128+p]
    def wt(W):
        return np.ascontiguousarray(
            W.T.reshape(NCH, P, D).transpose(1, 0, 2)).astype(BF16)

    WcT = wt(Wc)
    WaT = wt(Wa)
    vdiag = np.zeros((P, NCH, 2 * L2 - 1), dtype=BF16)
    vdiag[:, :, L2 - 1] = V.reshape(NCH, P).T.astype(BF16)

    in_maps = []
    for k in range(NCORES):
        ctx_s = context[NB * k:NB * (k + 1)]   # (NB, L1, D)
        asp_s = aspect[NB * k:NB * (k + 1)]    # (NB, L2, D)
        # [b, p, c, i] = ctx[b, i, c*128+p]
        ctxT = np.ascontiguousarray(
            ctx_s.transpose(0, 2, 1).reshape(NB, NCH, P, L1).transpose(0, 2, 1, 3)
        ).astype(BF16)
        aspT = np.ascontiguousarray(
            asp_s.transpose(0, 2, 1).reshape(NB, NCH, P, L2).transpose(0, 2, 1, 3)
        ).astype(BF16)
        in_maps.append({
            "ctxT": ctxT,
            "aspT": aspT,
            "asp": asp_s.astype(BF16),
            "WcT": WcT,
            "WaT": WaT,
            "vdiag": vdiag,
        })
    return in_maps


def run(inputs, trace=False, trace_kwargs=None, tmpdir=None):
    """Run on all 8 cores. Returns (full_output, BassKernelResults)."""
    from concourse.bass_utils import run_bass_kernel_spmd

    nc = _get_nc()
    in_maps = _shard_inputs(**inputs)
    res = run_bass_kernel_spmd(
        nc, in_maps, core_ids=list(range(NCORES)),
        trace=trace, trace_kwargs=trace_kwargs or {}, tmpdir=tmpdir)
    out = np.concatenate([res.results[k]["out"] for k in range(NCORES)], axis=0)
    return out.astype(np.float32), res


def kernel(**inputs):
    return run(inputs)[0]


# revision 14
# speedup vs baseline: 4.3331x; 1.0234x over previous
"""Bahdanau additive attention kernel for 8 TRN2 NeuronCores.

Reference math (per batch b):
    c = context @ Wc.T                     (L1, D)
    a = aspect  @ Wa.T                     (L2, D)
    scores[i,j] = sum_d V[d] * tanh(c[i,d] + a[j,d])
    alpha = softmax_j(scores)
    out = alpha @ aspect                   (L1, D)

Sharding: data-parallel over batch, 4 batches per core, no collectives.

Device mapping (per batch):
  - projections cT = Wc @ ctxT and aT = Wa @ aspT with the contraction (input
    feature) dim on partitions; host pre-transposes all operands so no
    on-device transposes are needed.
  - main loop over 4 e-chunks (output feature dim, 128 partitions each):
      DVE tensor_scalar_add broadcasts aT[:, j] over the i dim  (bf16, 4x mode)
      ACT computes one big tanh over a [128, JH*256] slab
      PE reduces against V with a "sliding diagonal" stationary operand:
        lhsT = vdiag[:, 63-j : 127-j]  (V in column j, zeros elsewhere)
        each matmul accumulates scoresT[j, :] into a [64, 256] PSUM tile
  - epilogue: exp on ACT (no max subtraction needed: |scores| <= sum|V| ~ 18),
    row sums + alpha@aspect as K=64 matmuls, final normalize by reciprocal
    on the way out.
"""

import numpy as np
import ml_dtypes

B, L1, L2, D = 32, 256, 64, 512
NCORES = 8
NB = B // NCORES          # batches per core
P = 128                   # partitions
NCH = D // P              # feature chunks (4)
JH = 32                   # j-slab size (2 slabs of 32 per e-chunk)
NI = L1 // P              # i chunks (2)

BF16 = ml_dtypes.bfloat16

_CACHE = {}


def _build():
    import concourse.bass as bass
    import concourse.tile as tile
    from concourse import bacc, mybir

    f32 = mybir.dt.float32
    bf16 = mybir.dt.bfloat16
    AFT = mybir.ActivationFunctionType
    ts = bass.ts

    nc = bacc.Bacc("TRN2", target_bir_lowering=False, debug=False,
                   num_devices=NCORES)

    ctxT_d = nc.dram_tensor("ctxT", [NB, P, NCH, L1], bf16, kind="ExternalInput")
    aspT_d = nc.dram_tensor("aspT", [NB, P, NCH, L2], bf16, kind="ExternalInput")
    asp_d = nc.dram_tensor("asp", [NB, L2, D], bf16, kind="ExternalInput")
    WcT_d = nc.dram_tensor("WcT", [P, NCH, D], bf16, kind="ExternalInput")
    WaT_d = nc.dram_tensor("WaT", [P, NCH, D], bf16, kind="ExternalInput")
    vdiag_d = nc.dram_tensor("vdiag", [P, NCH, 2 * L2 - 1], bf16, kind="ExternalInput")
    out_d = nc.dram_tensor("out", [NB, L1, D], f32, kind="ExternalOutput")

    with tile.TileContext(nc) as tc:
        with (
            tc.tile_pool(name="wpool", bufs=1) as wpool,
            tc.tile_pool(name="inpool", bufs=2) as inpool,
            tc.tile_pool(name="proj", bufs=1, space=bass.MemorySpace.PSUM) as projp,
            tc.tile_pool(name="ctpool", bufs=2) as ctpool,
            tc.tile_pool(name="slab", bufs=3) as slabp,
            tc.tile_pool(name="scores", bufs=2, space=bass.MemorySpace.PSUM) as scoresp,
            tc.tile_pool(name="eps", bufs=2, space=bass.MemorySpace.PSUM) as epsp,
            tc.tile_pool(name="small", bufs=2) as smallp,
            tc.tile_pool(name="epool", bufs=2) as epool,
            tc.tile_pool(name="outp", bufs=2) as outpool,
        ):
            WcT = wpool.tile([P, NCH, D], bf16)
            WaT = wpool.tile([P, NCH, D], bf16)
            vdiag = wpool.tile([P, NCH, 2 * L2 - 1], bf16)
            ones = wpool.tile([L2, 1], bf16)
            scratch = wpool.tile([L2, 1], bf16)
            nc.gpsimd.memset(ones[:], 1.0)
            # warm up the ACT table set (tanh/exp share one set) during DMAs
            nc.scalar.activation(scratch[:], ones[:], AFT.Tanh)

            for b in range(NB):
                ctxT = inpool.tile([P, NCH, L1], bf16, tag="ctx")
                aspT = inpool.tile([P, NCH, L2], bf16, tag="aspT")
                asp = inpool.tile([L2, D], bf16, tag="asp")
                if b == 0:
                    # startup: two HWDGE issue queues in parallel (ACT is idle
                    # here) so the first projection's operands land ASAP
                    nc.sync.dma_start(WcT[:], WcT_d[:])
                    nc.scalar.dma_start(ctxT[:], ctxT_d[b])
                    nc.sync.dma_start(aspT[:], aspT_d[b])
                    nc.scalar.dma_start(WaT[:], WaT_d[:])
                    nc.sync.dma_start(vdiag[:], vdiag_d[:])
                    nc.scalar.dma_start(asp[:], asp_d[b])
                else:
                    nc.sync.dma_start(ctxT[:], ctxT_d[b])
                    nc.sync.dma_start(aspT[:], aspT_d[b])
                    nc.sync.dma_start(asp[:], asp_d[b])

                # projections: cT[e,i] = sum_d WcT[d,e] * ctxT[d,i]
                cT = ctpool.tile([P, NCH, L1], bf16, tag="ct")
                aT = ctpool.tile([P, NCH, L2], f32, tag="at")
                psc = projp.tile([P, NCH, L1], f32, tag="projc")
                psa = projp.tile([P, NCH, L2], f32, tag="proja")
                scores = scoresp.tile([L2, L1], f32)

                def proj_m(m):
                    for c in range(NCH):
                        nc.tensor.matmul(psc[:, m, :], WcT[:, c, ts(m, P)],
                                         ctxT[:, c, :],
                                         start=(c == 0), stop=(c == NCH - 1))
                    for c in range(NCH):
                        nc.tensor.matmul(psa[:, m, :], WaT[:, c, ts(m, P)],
                                         aspT[:, c, :],
                                         start=(c == 0), stop=(c == NCH - 1))

                def slabs_m(m):
                    # ramped sub-slabs at the very start / end of the kernel
                    # cut ACT idle (startup latency, V-matmul drain tail)
                    if b == 0 and m == 0:
                        subs = [8, 8, 16, 32]
                    elif b == NB - 1 and m == NCH - 1:
                        subs = [32, 16, 8, 8]
                    else:
                        subs = [JH] * (L2 // JH)
                    jj = 0
                    for sub in subs:
                        tmp = slabp.tile([P, sub, L1], bf16, tag="tmp")
                        for j in range(sub):
                            nc.vector.tensor_scalar_add(
                                tmp[:, j, :], cT[:, m, :], aT[:, m, jj + j:jj + j + 1])
                        tha = slabp.tile([P, sub, L1], bf16, tag="tanh")
                        nc.scalar.activation(tha[:], tmp[:], AFT.Tanh)
                        for j in range(sub):
                            nc.tensor.matmul(
                                scores[:],
                                vdiag[:, m, L2 - 1 - (jj + j):2 * L2 - 1 - (jj + j)],
                                tha[:, j, :],
                                start=(m == 0 and jj + j == 0),
                                stop=(m == NCH - 1 and jj + j == L2 - 1))
                        jj += sub

                if b == 0:
                    # interleave per m-chunk: the first slab only depends on
                    # chunk-0 projections, not the whole batch's
                    for m in range(NCH):
                        proj_m(m)
                        nc.vector.tensor_copy(cT[:, m, :], psc[:, m, :])
                        nc.vector.tensor_copy(aT[:, m, :], psa[:, m, :])
                        slabs_m(m)
                else:
                    for m in range(NCH):
                        proj_m(m)
                    nc.vector.tensor_copy(cT[:], psc[:])
                    nc.vector.tensor_copy(aT[:], psa[:])
                    for m in range(NCH):
                        slabs_m(m)

                # softmax (over j = partitions of scores) + weighted sum
                E = epool.tile([L2, L1], bf16)
                for i in range(NI):
                    nc.scalar.activation(E[:, ts(i, P)], scores[:, ts(i, P)], AFT.Exp)
                    sums = epsp.tile([P, 1], f32, tag="eps")
                    nc.tensor.matmul(sums[:], E[:, ts(i, P)], ones[:])
                    recip = smallp.tile([P, 1], f32)
                    nc.vector.reciprocal(recip[:], sums[:])
                    op = epsp.tile([P, D], f32, tag="eps")
                    nc.tensor.matmul(op[:], E[:, ts(i, P)], asp[:])
                    osb = outpool.tile([P, D], f32)
                    nc.vector.tensor_scalar_mul(osb[:], op[:], recip[:])
                    nc.sync.dma_start(out_d[b, ts(i, P), :], osb[:])

    nc.compile()
    return nc


def _get_nc():
    if "nc" not in _CACHE:
        _CACHE["nc"] = _build()
    return _CACHE["nc"]


def _shard_inputs(context, aspect, Wc, Wa, V):
    """Host-side preprocessing: shard over batch, transpose + cast to bf16."""
    context = np.asarray(context)
    aspect = np.asarray(aspect)
    Wc = np.asarray(Wc)
    Wa = np.asarray(Wa)
    V = np.asarray(V)

    # [p, c, e] = W[e, c*128+p]
    def wt(W):
        return np.ascontiguousarray(
            W.T.reshape(NCH, P, D).transpose(1, 0, 2)).astype(BF16)

    WcT = wt(Wc)
    WaT = wt(Wa)
    vdiag = np.zeros((P, NCH, 2 * L2 - 1), dtype=BF16)
    vdiag[:, :, L2 - 1] = V.reshape(NCH, P).T.astype(BF16)

    in_maps = []
    for k in range(NCORES):
        ctx_s = context[NB * k:NB * (k + 1)]   # (NB, L1, D)
        asp_s = aspect[NB * k:NB * (k + 1)]    # (NB, L2, D)
        # [b, p, c, i] = ctx[b, i, c*128+p]
        ctxT = np.ascontiguousarray(
            ctx_s.transpose(0, 2, 1).reshape(NB, NCH, P, L1).transpose(0, 2, 1, 3)
        ).astype(BF16)
        aspT = np.ascontiguousarray(
            asp_s.transpose(0, 2, 1).reshape(NB, NCH, P, L2).transpose(0, 2, 1, 3)
        ).astype(BF16)
        in_maps.append({
            "ctxT": ctxT,
            "aspT": aspT,
            "asp": asp_s.astype(BF16),
            "WcT": WcT,
            "WaT": WaT,
            "vdiag": vdiag,
        })
    return in_maps


def run(inputs, trace=False, trace_kwargs=None, tmpdir=None):
    """Run on all 8 cores. Returns (full_output, BassKernelResults)."""
    from concourse.bass_utils import run_bass_kernel_spmd

    nc = _get_nc()
    in_maps = _shard_inputs(**inputs)
    res = run_bass_kernel_spmd(
        nc, in_maps, core_ids=list(range(NCORES)),
        trace=trace, trace_kwargs=trace_kwargs or {}, tmpdir=tmpdir)
    out = np.concatenate([res.results[k]["out"] for k in range(NCORES)], axis=0)
    return out.astype(np.float32), res


def kernel(**inputs):
    return run(inputs)[0]
